# revision 21
# baseline (speedup 1.0000x reference)
"""MixedDecoder (dense MoE blend) Trainium2 kernel, fp8 edition.

Data-parallel over 8 NeuronCores: batch 512 -> 64 rows/core, weights
replicated. The mixed layer out = sum_e coeff[:,e]*(x @ W_e) + coeff @ b is
one PSUM-accumulated matmul over the concatenated K axis, with the moving
weights streamed from HBM.

Precision plan (verified vs reference in fp emulation, rel_err ~1e-2):
 - layer0: weights + scaled inputs fp8e4 (DoubleRow matmuls, 2 K-tiles each)
 - layer1: h-rows fp8e4 DoubleRow, z-rows bf16
 - layer2 + gating: bf16
Weights are pre-scaled x64 and scaled inputs x16 on host/device; the PSUM is
1024x true and gets unscaled in the ELU (ACT scale=1/1024) / output copy.

Expert tails share K-tiles across experts (stacked on partitions), so there
is no K padding: L0 = 5 DR matmuls, L1 = 16 DR + 4 bf16, L2 = 36 bf16.
Per-expert coefficient planes come from 3 selection matmuls against a
mask-multiplied coefficient tensor, built on the fly from the gating output.

Engine choreography: ELU pieces split un(DVE)/relu,exp(ACT)/combine(GPSIMD),
transposes+xscale feed the next layer chunk-by-chunk so layer-N matmuls
overlap the seam, and a few throwaway DoubleRow matmuls bridge the PE's
p-state ramp across the first seam.
"""

import numpy as np
import ml_dtypes

import concourse.bass as bass
import concourse.tile as tile
from concourse import bacc, mybir
from concourse import bass_utils

BF16 = mybir.dt.bfloat16
FP8 = mybir.dt.float8e4
F32 = mybir.dt.float32
AF = mybir.ActivationFunctionType
OP = mybir.AluOpType
DR = mybir.MatmulPerfMode.DoubleRow

B, L, FS, H, E = 512, 64, 96, 512, 8
IN = L + FS          # 160
INTER = L + H        # 576
OUT = FS             # 96
NCORES = 8
BL = B // NCORES     # 64 batch rows per core

WS = 64.0            # weight scale
XS = 16.0            # scaled-input scale
INV = 1.0 / (WS * XS)

_nbf = ml_dtypes.bfloat16
_nf8 = ml_dtypes.float8_e4m3

# gpack column layout (bf16 [128, 656]):
_GP_ZCT0 = 0      # [128, 64] zc.T rows 0:128
_GP_ZDUP = 64     # [128, 64] z.T rows 0:64 duplicated twice on partitions
_GP_ZT4 = 128     # [128, 64] zc.T rows 128:160 stacked x4
_GP_ZCT1 = 192    # [32, 64]  zc.T rows 128:160
_GP_GW0M = 256    # [128, 64]
_GP_GW0T = 320    # [32, 64]
_GP_GW1 = 384    # [64, 64]
_GP_ID = 448      # [64, 64] identity
_GP_GW2 = 512     # [64, 8]
_GP_GB0 = 520     # [1, 64]
_GP_GB1 = 584     # [1, 64]
_GP_GB2 = 648     # [1, 8]
_GP_ID16 = 656    # [64, 64] identity, fp16 bits stored in bf16 slots
_GP_COLS = 720

# aux (bf16 [8, 2400]) layout:
_AX_SELT = 0      # [8, 128] all ones
_AX_SELP = 128    # [8, 128] (e%2) == (p>=64)
_AX_SELQ = 256    # [8, 128] (e%4) == p//32
_AX_MASK = 384    # [8, 14, 64] mask*16, b-expanded
_AX_BC0 = 1280    # [8, 512] b0*1024
_AX_BC1 = 1792    # [8, 512] b1*1024
_AX_BC2 = 2304    # [8, 96]  b2*1024
_AX_COLS = 2400


def _build():
    nc = bacc.Bacc("TRN2", target_bir_lowering=False, debug=False,
                   num_devices=NCORES)

    def din(name, shape, dtype):
        return nc.dram_tensor(name, list(shape), dtype,
                              kind="ExternalInput").ap()

    gpack = din("gpack", [128, _GP_COLS], BF16)
    aux = din("aux", [E, _AX_COLS], BF16)
    w0q = din("w0q", [128, 5 * 2 * 512], FP8)
    w1q = din("w1q", [128, E * 4 * 512], FP8)
    wb = din("wb", [128, 4 * 512 + E * 4 * 96 + 4 * 96], BF16)

    out_d = nc.dram_tensor("out", [BL, OUT], F32, kind="ExternalOutput").ap()

    with tile.TileContext(nc) as tc:
        with (
            tc.tile_pool(name="const", bufs=1) as cpool,
            tc.tile_pool(name="w", bufs=1) as wpool,
            tc.tile_pool(name="x", bufs=1) as xpool,
            tc.tile_pool(name="act", bufs=2) as apool,
            tc.tile_pool(name="psg", bufs=2, space="PSUM") as psg,
            tc.tile_pool(name="psm", bufs=2, space="PSUM") as psm,
            tc.tile_pool(name="pss", bufs=1, space="PSUM") as pss,
            tc.tile_pool(name="pso", bufs=1, space="PSUM") as pso,
        ):
            # ---- bulk loads: order matches consumption order ----
            gp = cpool.tile([128, _GP_COLS], BF16, tag="gp")
            nc.sync.dma_start(gp[:], gpack[:])
            ax = cpool.tile([E, _AX_COLS], BF16, tag="ax")
            nc.sync.dma_start(ax[:], aux[:])
            w0t = wpool.tile([128, 5, 2, 512], FP8, tag="w0")
            nc.sync.dma_start(w0t[:].rearrange("p a b c -> p (a b c)"), w0q[:])
            w1t = wpool.tile([128, 2, E, 2, 512], FP8, tag="w1")
            nc.sync.dma_start(w1t[:, 0].rearrange("p b c d -> p (b c d)"),
                              w1q[:, 0:8192])
            wbt = wpool.tile([128, 5504], BF16, tag="wb")
            nc.sync.dma_start(wbt[:], wb[:])
            nc.sync.dma_start(w1t[:, 1].rearrange("p b c d -> p (b c d)"),
                              w1q[:, 8192:16384])

            # views
            zcT0 = gp[:, _GP_ZCT0:_GP_ZCT0 + 64]
            zdup = gp[:, _GP_ZDUP:_GP_ZDUP + 64]
            zt4 = gp[:, _GP_ZT4:_GP_ZT4 + 64]
            zcT1 = gp[0:32, _GP_ZCT1:_GP_ZCT1 + 64]
            gb0v = gp[0:1, _GP_GB0:_GP_GB0 + 64]
            gb1v = gp[0:1, _GP_GB1:_GP_GB1 + 64]
            gb2v = gp[0:1, _GP_GB2:_GP_GB2 + 8]
            gw0m = gp[:, _GP_GW0M:_GP_GW0M + 64]
            gw0t = gp[0:32, _GP_GW0T:_GP_GW0T + 64]
            gw1v = gp[0:64, _GP_GW1:_GP_GW1 + 64]
            identv = gp[0:64, _GP_ID:_GP_ID + 64]
            identv16 = gp[0:64, _GP_ID16:_GP_ID16 + 64].bitcast(
                mybir.dt.float16)
            gw2v = gp[0:64, _GP_GW2:_GP_GW2 + E]

            selT = ax[:, _AX_SELT:_AX_SELT + 128]
            selP = ax[:, _AX_SELP:_AX_SELP + 128]
            selQ = ax[:, _AX_SELQ:_AX_SELQ + 128]
            maskv = ax[:, _AX_MASK:_AX_MASK + 896].rearrange(
                "p (a b) -> p a b", a=14)
            bc0 = ax[:, _AX_BC0:_AX_BC0 + 512]
            bc1 = ax[:, _AX_BC1:_AX_BC1 + 512]
            bc2 = ax[:, _AX_BC2:_AX_BC2 + 96]

            w1z = wbt[:, 0:2048].rearrange("p (a b) -> p a b", a=4)
            w2h = wbt[:, 2048:5120].rearrange("p (a b c) -> p a b c", a=E, b=4)
            w2z = wbt[:, 5120:5504].rearrange("p (a b) -> p a b", a=4)

            ones_t = cpool.tile([1, BL], BF16, tag="ones")
            nc.gpsimd.memset(ones_t[:], 1.0)

            # ---- ELU pieces, ACT-heavy: un = relu(-x*s); ex = exp(-un)
            # = exp(min(x*s,0)); rl = relu(x*s); out = (ex - 1) + rl [DVE]
            def elu(dst_bf16, src_psum, shape, s=1.0, comb=None):
                rl = apool.tile(shape, F32, tag="elu_rl", bufs=4)
                un = apool.tile(shape, F32, tag="elu_un", bufs=4)
                ex = apool.tile(shape, F32, tag="elu_ex", bufs=4)
                nc.scalar.activation(un[:], src_psum, AF.Relu, scale=-s)
                nc.scalar.activation(ex[:], un[:], AF.Exp, scale=-1.0)
                nc.scalar.activation(rl[:], src_psum, AF.Relu, scale=s)
                nc.vector.scalar_tensor_tensor(dst_bf16, ex[:], -1.0, rl[:],
                                               OP.add, OP.add)

            # selection-plane PSUM doubles as scratch for warm-up matmuls
            sa_ps = pss.tile([128, E, BL], F32, tag="SA")

            def warm(n, src=None):
                # throwaway DR matmuls; keep the PE p-state ramp alive while
                # other engines run serial chains (results never read)
                for i in range(n):
                    nc.tensor.matmul(sa_ps[0:64, :, :], w0t[:, i % 4, :, 0:64],
                                     w0t[:, i % 4, :, :], start=True,
                                     stop=True, perf_mode=DR,
                                     skip_group_check=True)

            def warm_g(n):
                # bf16 warm variant usable as soon as gpack has landed
                for i in range(n):
                    nc.tensor.matmul(sa_ps[0:64, :, :], gw0m,
                                     gp[:, 0:512], start=True, stop=True,
                                     skip_group_check=True)

            # ---- gating ----
            g1ps = psg.tile([64, 64], F32, tag="gps", bufs=1)
            nc.tensor.matmul(g1ps[:], gb0v, ones_t[:], start=True, stop=False)
            nc.tensor.matmul(g1ps[:], gw0m, zcT0, start=False, stop=False)
            nc.tensor.matmul(g1ps[:], gw0t, zcT1, start=False, stop=True)
            g1_t = apool.tile([64, 64], BF16, tag="g1")
            elu(g1_t[:], g1ps[:], [64, 64])
            warm_g(2)

            g2ps = psg.tile([64, 64], F32, tag="gps", bufs=1)
            nc.tensor.matmul(g2ps[:], gb1v, ones_t[:], start=True, stop=False)
            nc.tensor.matmul(g2ps[:], gw1v, g1_t[:], start=False, stop=True)
            g2_t = apool.tile([64, 64], BF16, tag="g2")
            elu(g2_t[:], g2ps[:], [64, 64])
            warm_g(2)

            lgps = psg.tile([64, E], F32, tag="gps", bufs=1)
            nc.tensor.matmul(lgps[:], ones_t[:], gb2v, start=True, stop=False)
            nc.tensor.matmul(lgps[:], g2_t[:], gw2v, start=False, stop=True)
            warm(3)

            exps_t = apool.tile([64, E], F32, tag="exps")
            se_t = apool.tile([64, 1], F32, tag="se")
            nc.scalar.activation(exps_t[:], lgps[:], AF.Exp, accum_out=se_t[:])
            rec_t = apool.tile([64, 1], F32, tag="rec")
            nc.vector.reciprocal(rec_t[:], se_t[:])
            coeff_t = apool.tile([64, E], BF16, tag="coeff")
            nc.vector.tensor_scalar(coeff_t[:], exps_t[:], rec_t[:], None,
                                    OP.mult)

            # coeff transpose [64,8] -> [8,64] (PSUM, read in place)
            ctps = psg.tile([E, 64], BF16, tag="ctps", bufs=1)
            nc.tensor.matmul(ctps[:], coeff_t[:], identv, is_transpose=True,
                             start=True, stop=True)
            warm(2)

            # masked coeff tensor: Ball[e, j, b] = coeffT[e,b] * mask[e,j]
            ball_t = cpool.tile([E, 14, BL], BF16, tag="ball")
            nc.vector.tensor_tensor(
                ball_t[:], ctps[:].unsqueeze(1).broadcast_to((E, 14, BL)),
                maskv, OP.mult)
            coeffT_t = cpool.tile([E, BL], BF16, tag="coeffT")
            nc.vector.tensor_copy(coeffT_t[:], ctps[:])

            # selection matmuls -> per-partition coefficient planes (x16)
            nc.tensor.matmul(sa_ps[:], selT, ball_t[:, 0:8, :],
                             start=True, stop=True)
            sb_ps = pss.tile([128, 6, BL], F32, tag="SB")
            nc.tensor.matmul(sb_ps[:, 0:4, :], selP, ball_t[:, 8:12, :],
                             start=True, stop=True)
            nc.tensor.matmul(sb_ps[:, 4:6, :], selQ, ball_t[:, 12:14, :],
                             start=True, stop=True)

            # scaled inputs straight from the PSUM selection results
            x0m_t = xpool.tile([128, E, BL], FP8, tag="x0m")
            nc.vector.tensor_tensor(
                x0m_t[:, 0:4, :], zcT0.unsqueeze(1).broadcast_to((128, 4, BL)),
                sa_ps[:, 0:4, :], OP.mult)
            nc.vector.tensor_tensor(
                x0m_t[:, 4:8, :], zcT0.unsqueeze(1).broadcast_to((128, 4, BL)),
                sa_ps[:, 4:8, :], OP.mult)
            x0tail_t = cpool.tile([128, 2, BL], FP8, tag="x0tail")
            nc.vector.tensor_tensor(
                x0tail_t[:], zt4.unsqueeze(1).broadcast_to((128, 2, BL)),
                sb_ps[:, 4:6, :], OP.mult)
            xz_t = cpool.tile([128, 4, BL], BF16, tag="xz")
            nc.vector.tensor_tensor(
                xz_t[:], zdup.unsqueeze(1).broadcast_to((128, 4, BL)),
                sb_ps[:, 0:4, :], OP.mult)

            # bf16 coefficient planes for the seam scalings; copied before
            # any warm matmul can stomp the selection PSUM
            S_t = cpool.tile([128, E, BL], BF16, tag="S")
            nc.vector.tensor_copy(S_t[:], sa_ps[:])

            x1h_t = xpool.tile([128, 4, E, BL], FP8, tag="x1h")
            x2h_t = xpool.tile([128, 4, E, BL], BF16, tag="x2h")
            warm(2)

            # ---- layer 0: bias + 4 main DR pairs + 1 tail DR ----
            l0ps = psm.tile([64, H], F32, tag="lps")
            nc.tensor.matmul(l0ps[:], coeffT_t[:], bc0, start=True, stop=False)
            for g in range(4):
                nc.tensor.matmul(l0ps[:], x0m_t[:, 2 * g:2 * g + 2, :],
                                 w0t[:, g, :, :], start=False, stop=False,
                                 perf_mode=DR)
            nc.tensor.matmul(l0ps[:], x0tail_t[:], w0t[:, 4, :, :],
                             start=False, stop=True, perf_mode=DR)

            # ---- layer 1 bias + z-part (runs on PE during seam 0) ----
            l1ps = psm.tile([64, H], F32, tag="lps")
            nc.tensor.matmul(l1ps[:], coeffT_t[:], bc1, start=True, stop=False)
            for j in range(4):
                nc.tensor.matmul(l1ps[:], xz_t[:, j, :], w1z[:, j, :],
                                 start=False, stop=False)

            # ---- seam helper: ELU chunk -> transpose -> scaled next input
            def seam_chunk(l_ps, m, hb, hTp, xt_next):
                sl = slice(128 * m, 128 * (m + 1))
                elu(hb[:, sl], l_ps[:, sl], [64, 128], s=INV)
                nc.tensor.matmul(hTp[:, m, :], hb[:, sl], identv,
                                 is_transpose=True, start=True, stop=True)
                nc.vector.tensor_tensor(
                    xt_next[:, m, :, :],
                    hTp[:, m, :].unsqueeze(1).broadcast_to((128, E, BL)),
                    S_t[:], OP.mult)

            hb0 = apool.tile([64, 512], BF16, tag="hb")
            hTp0 = psm.tile([128, 4, BL], BF16, tag="hTp", bufs=1)
            seam_chunk(l0ps, 0, hb0, hTp0, x1h_t)
            seam_chunk(l0ps, 1, hb0, hTp0, x1h_t)

            warm(2)

            # ---- layer 1 h-part, P-outer, transposes slotted mid-block ----
            for e in range(4):
                nc.tensor.matmul(l1ps[:], x1h_t[:, 0:2, e, :],
                                 w1t[:, 0, e, :, :], start=False,
                                 stop=False, perf_mode=DR)
            warm(2)
            seam_chunk(l0ps, 2, hb0, hTp0, x1h_t)
            for e in range(4, E):
                nc.tensor.matmul(l1ps[:], x1h_t[:, 0:2, e, :],
                                 w1t[:, 0, e, :, :], start=False,
                                 stop=False, perf_mode=DR)
            warm(2)
            seam_chunk(l0ps, 3, hb0, hTp0, x1h_t)
            for e in range(E):
                nc.tensor.matmul(l1ps[:], x1h_t[:, 2:4, e, :],
                                 w1t[:, 1, e, :, :], start=False,
                                 stop=(e == E - 1), perf_mode=DR)

            # ---- layer 2 bias + z-part (runs on PE during seam 1) ----
            l2ps = pso.tile([BL, OUT], F32, tag="ops")
            nc.tensor.matmul(l2ps[:], coeffT_t[:], bc2, start=True, stop=False)
            for j in range(4):
                nc.tensor.matmul(l2ps[:], xz_t[:, j, :], w2z[:, j, :],
                                 start=False, stop=False)

            warm(5)

            # ---- seam 1 interleaved with layer 2 h-part (t-outer) ----
            hb1 = apool.tile([64, 512], BF16, tag="hb")
            hTp1 = psm.tile([128, 4, BL], BF16, tag="hTp", bufs=1)
            for t in range(4):
                seam_chunk(l1ps, t, hb1, hTp1, x2h_t)
                for e in range(E):
                    nc.tensor.matmul(l2ps[:], x2h_t[:, t, e, :],
                                     w2h[:, e, t, :], start=False,
                                     stop=(t == 3 and e == E - 1))
                if t < 3:
                    warm(1)

            out_t = apool.tile([BL, OUT], F32, tag="out_sb")
            nc.vector.tensor_scalar(out_t[:], l2ps[:], INV, None, OP.mult)
            nc.scalar.dma_start(out_d[:], out_t[:])

    nc.compile()
    return nc


_NC_CACHE = None


def _get_nc():
    global _NC_CACHE
    if _NC_CACHE is None:
        _NC_CACHE = _build()
    return _NC_CACHE


def _host_prep(z, c, gw0, gb0, gw1, gb1, gw2, gb2, w0, b0, w1, b1, w2, b2):
    bf = lambda a: np.ascontiguousarray(a).astype(_nbf)
    f8 = lambda a: np.ascontiguousarray(a).astype(_nf8)
    z, c = np.asarray(z), np.asarray(c)
    gw0, gw1, gw2 = np.asarray(gw0), np.asarray(gw1), np.asarray(gw2)
    w0, w1, w2 = np.asarray(w0) * WS, np.asarray(w1) * WS, np.asarray(w2) * WS
    b0, b1, b2 = np.asarray(b0), np.asarray(b1), np.asarray(b2)

    # w0q [128, 5, 2, 512]
    w0q = np.zeros((128, 5, 2, 512), dtype=np.float32)
    for g in range(4):
        for i in range(2):
            w0q[:, g, i, :] = w0[2 * g + i, 0:128, :]
    for j in range(2):
        w0q[:, 4, j, :] = w0[4 * j:4 * j + 4, 128:160, :].reshape(128, 512)

    # w1q [128, 2, E, 2, 512] (h-rows, P-major), w1z [128, 4, 512] (z-rows)
    w1q = (w1[:, 64:576, :].reshape(E, 2, 2, 128, H)
           .transpose(3, 1, 0, 2, 4))
    w1zp = (w1[:, 0:64, :].reshape(4, 2 * 64, H)
            .transpose(1, 0, 2))                       # [128, 4, 512]
    w2hp = (w2[:, 64:576, :].reshape(E, 4, 128, OUT)
            .transpose(2, 0, 1, 3))                    # [128, 8, 4, 96]
    w2zp = (w2[:, 0:64, :].reshape(4, 2 * 64, OUT)
            .transpose(1, 0, 2))                       # [128, 4, 96]
    wbp = np.concatenate([w1zp.reshape(128, -1), w2hp.reshape(128, -1),
                          w2zp.reshape(128, -1)], axis=1)

    # aux [8, 2400]
    aux = np.zeros((E, _AX_COLS), dtype=np.float32)
    ei = np.arange(E)[:, None]
    pi = np.arange(128)[None, :]
    aux[:, _AX_SELT:_AX_SELT + 128] = 1.0
    aux[:, _AX_SELP:_AX_SELP + 128] = ((ei % 2) == (pi >= 64))
    aux[:, _AX_SELQ:_AX_SELQ + 128] = ((ei % 4) == (pi // 32))
    mask = np.zeros((E, 14), dtype=np.float32)
    mask[:, 0:8] = np.eye(E) * XS
    ji = np.arange(4)[None, :]
    mask[:, 8:12] = ((ei // 2) == ji) * XS
    mask[:, 12:14] = ((ei // 4) == np.arange(2)[None, :]) * XS
    aux[:, _AX_MASK:_AX_MASK + 896] = np.repeat(
        mask, BL, axis=1).reshape(E, 896)
    aux[:, _AX_BC0:_AX_BC0 + 512] = b0 * (WS * XS)
    aux[:, _AX_BC1:_AX_BC1 + 512] = b1 * (WS * XS)
    aux[:, _AX_BC2:_AX_BC2 + 96] = b2 * (WS * XS)

    gp_base = np.zeros((128, _GP_COLS), dtype=np.float32)
    gp_base[:, _GP_GW0M:_GP_GW0M + 64] = gw0[0:128]
    gp_base[0:32, _GP_GW0T:_GP_GW0T + 64] = gw0[128:160]
    gp_base[0:64, _GP_GW1:_GP_GW1 + 64] = gw1
    gp_base[0:64, _GP_GW2:_GP_GW2 + E] = gw2
    gp_base[0, _GP_GB0:_GP_GB0 + 64] = gb0
    gp_base[0, _GP_GB1:_GP_GB1 + 64] = gb1
    gp_base[0, _GP_GB2:_GP_GB2 + 8] = gb2
    gp_base[0:64, _GP_ID:_GP_ID + 64] = np.eye(64, dtype=np.float32)

    shared = {
        "aux": bf(aux),
        "w0q": f8(w0q.reshape(128, -1)),
        "w1q": f8(w1q.reshape(128, -1)),
        "wb": bf(wbp),
    }
    zc = np.concatenate([z, c], axis=1)  # [B, 160]
    in_maps = []
    for i in range(NCORES):
        gpi = gp_base.copy()
        zcT = zc[i * BL:(i + 1) * BL, :].T  # [160, 64]
        gpi[:, _GP_ZCT0:_GP_ZCT0 + 64] = zcT[0:128]
        gpi[0:64, _GP_ZDUP:_GP_ZDUP + 64] = zcT[0:64]
        gpi[64:128, _GP_ZDUP:_GP_ZDUP + 64] = zcT[0:64]
        gpi[:, _GP_ZT4:_GP_ZT4 + 64] = np.tile(zcT[128:160], (4, 1))
        gpi[0:32, _GP_ZCT1:_GP_ZCT1 + 64] = zcT[128:160]
        gpb = bf(gpi)
        gpb[0:64, _GP_ID16:_GP_ID16 + 64] = np.eye(
            64, dtype=np.float16).view(_nbf)
        m = dict(shared)
        m["gpack"] = gpb
        in_maps.append(m)
    return in_maps


def kernel(**inputs):
    nc = _get_nc()
    in_maps = _host_prep(**inputs)
    res = bass_utils.run_bass_kernel_spmd(nc, in_maps,
                                          core_ids=list(range(NCORES)))
    return np.concatenate([r["out"] for r in res.results], axis=0)


# revision 23
# speedup vs baseline: 1.0398x; 1.0398x over previous
"""MixedDecoder (dense MoE blend) Trainium2 kernel, fp8 edition.

Data-parallel over 8 NeuronCores: batch 512 -> 64 rows/core, weights
replicated. The mixed layer out = sum_e coeff[:,e]*(x @ W_e) + coeff @ b is
one PSUM-accumulated matmul over the concatenated K axis, with the moving
weights streamed from HBM.

Precision plan (verified vs reference in fp emulation, rel_err ~1e-2):
 - layer0: weights + scaled inputs fp8e4 (DoubleRow matmuls, 2 K-tiles each)
 - layer1: h-rows fp8e4 DoubleRow, z-rows bf16
 - layer2 + gating: bf16
Weights are pre-scaled x64 and scaled inputs x16 on host/device; the PSUM is
1024x true and gets unscaled in the ELU (ACT scale=1/1024) / output copy.

Expert tails share K-tiles across experts (stacked on partitions), so there
is no K padding: L0 = 5 DR matmuls, L1 = 16 DR + 4 bf16, L2 = 36 bf16.
Per-expert coefficient planes come from 3 selection matmuls against a
mask-multiplied coefficient tensor, built on the fly from the gating output.

Engine choreography: ELU runs relu/exp on ACT with the combine on DVE;
transposes+xscale feed the next layer chunk-by-chunk so layer-N matmuls
overlap the seams (L1 in P-halves around the chunk transposes, L2 t-blocks
after each chunk), and the z-feature matmuls run during the seams.
"""

import numpy as np
import ml_dtypes

import concourse.bass as bass
import concourse.tile as tile
from concourse import bacc, mybir
from concourse import bass_utils

BF16 = mybir.dt.bfloat16
FP8 = mybir.dt.float8e4
F32 = mybir.dt.float32
AF = mybir.ActivationFunctionType
OP = mybir.AluOpType
DR = mybir.MatmulPerfMode.DoubleRow

B, L, FS, H, E = 512, 64, 96, 512, 8
IN = L + FS          # 160
INTER = L + H        # 576
OUT = FS             # 96
NCORES = 8
BL = B // NCORES     # 64 batch rows per core

WS = 64.0            # weight scale
XS = 16.0            # scaled-input scale
INV = 1.0 / (WS * XS)

_nbf = ml_dtypes.bfloat16
_nf8 = ml_dtypes.float8_e4m3

# gpack column layout (bf16 [128, 656]):
_GP_ZCT0 = 0      # [128, 64] zc.T rows 0:128
_GP_ZDUP = 64     # [128, 64] z.T rows 0:64 duplicated twice on partitions
_GP_ZT4 = 128     # [128, 64] zc.T rows 128:160 stacked x4
_GP_ZCT1 = 192    # [32, 64]  zc.T rows 128:160
_GP_GW0M = 256    # [128, 64]
_GP_GW0T = 320    # [32, 64]
_GP_GW1 = 384    # [64, 64]
_GP_ID = 448      # [64, 64] identity
_GP_GW2 = 512     # [64, 8]
_GP_GB0 = 520     # [1, 64]
_GP_GB1 = 584     # [1, 64]
_GP_GB2 = 648     # [1, 8]
_GP_COLS = 656

# aux (bf16 [8, 2400]) layout:
_AX_SELT = 0      # [8, 128] all ones
_AX_SELP = 128    # [8, 128] (e%2) == (p>=64)
_AX_SELQ = 256    # [8, 128] (e%4) == p//32
_AX_MASK = 384    # [8, 14, 64] mask*16, b-expanded
_AX_BC0 = 1280    # [8, 512] b0*1024
_AX_BC1 = 1792    # [8, 512] b1*1024
_AX_BC2 = 2304    # [8, 96]  b2*1024
_AX_COLS = 2400


def _build():
    nc = bacc.Bacc("TRN2", target_bir_lowering=False, debug=False,
                   num_devices=NCORES)

    def din(name, shape, dtype):
        return nc.dram_tensor(name, list(shape), dtype,
                              kind="ExternalInput").ap()

    gpack = din("gpack", [128, _GP_COLS], BF16)
    aux = din("aux", [E, _AX_COLS], BF16)
    w0q = din("w0q", [128, 5 * 2 * 512], FP8)
    w1q = din("w1q", [128, E * 4 * 512], FP8)
    wb = din("wb", [128, 4 * 512 + E * 4 * 96 + 4 * 96], BF16)

    out_d = nc.dram_tensor("out", [BL, OUT], F32, kind="ExternalOutput").ap()

    with tile.TileContext(nc) as tc:
        with (
            tc.tile_pool(name="const", bufs=1) as cpool,
            tc.tile_pool(name="w", bufs=1) as wpool,
            tc.tile_pool(name="x", bufs=1) as xpool,
            tc.tile_pool(name="act", bufs=2) as apool,
            tc.tile_pool(name="psg", bufs=2, space="PSUM") as psg,
            tc.tile_pool(name="psm", bufs=2, space="PSUM") as psm,
            tc.tile_pool(name="pss", bufs=1, space="PSUM") as pss,
            tc.tile_pool(name="pso", bufs=1, space="PSUM") as pso,
        ):
            # ---- bulk loads: order matches consumption order ----
            gp = cpool.tile([128, _GP_COLS], BF16, tag="gp")
            nc.sync.dma_start(gp[:], gpack[:])
            ax = cpool.tile([E, _AX_COLS], BF16, tag="ax")
            nc.sync.dma_start(ax[:], aux[:])
            w0t = wpool.tile([128, 5, 2, 512], FP8, tag="w0")
            nc.sync.dma_start(w0t[:].rearrange("p a b c -> p (a b c)"), w0q[:])
            w1t = wpool.tile([128, 2, E, 2, 512], FP8, tag="w1")
            nc.sync.dma_start(w1t[:, 0].rearrange("p b c d -> p (b c d)"),
                              w1q[:, 0:8192])
            wbt = wpool.tile([128, 5504], BF16, tag="wb")
            nc.sync.dma_start(wbt[:], wb[:])
            nc.sync.dma_start(w1t[:, 1].rearrange("p b c d -> p (b c d)"),
                              w1q[:, 8192:16384])

            # views
            zcT0 = gp[:, _GP_ZCT0:_GP_ZCT0 + 64]
            zdup = gp[:, _GP_ZDUP:_GP_ZDUP + 64]
            zt4 = gp[:, _GP_ZT4:_GP_ZT4 + 64]
            zcT1 = gp[0:32, _GP_ZCT1:_GP_ZCT1 + 64]
            gb0v = gp[0:1, _GP_GB0:_GP_GB0 + 64]
            gb1v = gp[0:1, _GP_GB1:_GP_GB1 + 64]
            gb2v = gp[0:1, _GP_GB2:_GP_GB2 + 8]
            gw0m = gp[:, _GP_GW0M:_GP_GW0M + 64]
            gw0t = gp[0:32, _GP_GW0T:_GP_GW0T + 64]
            gw1v = gp[0:64, _GP_GW1:_GP_GW1 + 64]
            identv = gp[0:64, _GP_ID:_GP_ID + 64]
            gw2v = gp[0:64, _GP_GW2:_GP_GW2 + E]

            selT = ax[:, _AX_SELT:_AX_SELT + 128]
            selP = ax[:, _AX_SELP:_AX_SELP + 128]
            selQ = ax[:, _AX_SELQ:_AX_SELQ + 128]
            maskv = ax[:, _AX_MASK:_AX_MASK + 896].rearrange(
                "p (a b) -> p a b", a=14)
            bc0 = ax[:, _AX_BC0:_AX_BC0 + 512]
            bc1 = ax[:, _AX_BC1:_AX_BC1 + 512]
            bc2 = ax[:, _AX_BC2:_AX_BC2 + 96]

            w1z = wbt[:, 0:2048].rearrange("p (a b) -> p a b", a=4)
            w2h = wbt[:, 2048:5120].rearrange("p (a b c) -> p a b c", a=E, b=4)
            w2z = wbt[:, 5120:5504].rearrange("p (a b) -> p a b", a=4)

            ones_t = cpool.tile([1, BL], BF16, tag="ones")
            nc.gpsimd.memset(ones_t[:], 1.0)

            # ---- ELU pieces, ACT-heavy: un = relu(-x*s); ex = exp(-un)
            # = exp(min(x*s,0)); rl = relu(x*s); out = (ex - 1) + rl [DVE]
            def elu(dst_bf16, src_psum, shape, s=1.0, comb=None):
                rl = apool.tile(shape, F32, tag="elu_rl", bufs=4)
                un = apool.tile(shape, F32, tag="elu_un", bufs=4)
                ex = apool.tile(shape, F32, tag="elu_ex", bufs=4)
                nc.scalar.activation(un[:], src_psum, AF.Relu, scale=-s)
                nc.scalar.activation(ex[:], un[:], AF.Exp, scale=-1.0)
                nc.scalar.activation(rl[:], src_psum, AF.Relu, scale=s)
                nc.vector.scalar_tensor_tensor(dst_bf16, ex[:], -1.0, rl[:],
                                               OP.add, OP.add)

            sa_ps = pss.tile([128, E, BL], F32, tag="SA")

            # ---- gating ----
            g1ps = psg.tile([64, 64], F32, tag="gps", bufs=1)
            nc.tensor.matmul(g1ps[:], gb0v, ones_t[:], start=True, stop=False)
            nc.tensor.matmul(g1ps[:], gw0m, zcT0, start=False, stop=False)
            nc.tensor.matmul(g1ps[:], gw0t, zcT1, start=False, stop=True)
            g1_t = apool.tile([64, 64], BF16, tag="g1")
            elu(g1_t[:], g1ps[:], [64, 64])
            warm(2)

            g2ps = psg.tile([64, 64], F32, tag="gps", bufs=1)
            nc.tensor.matmul(g2ps[:], gb1v, ones_t[:], start=True, stop=False)
            nc.tensor.matmul(g2ps[:], gw1v, g1_t[:], start=False, stop=True)
            g2_t = apool.tile([64, 64], BF16, tag="g2")
            elu(g2_t[:], g2ps[:], [64, 64])
            warm(2)

            lgps = psg.tile([64, E], F32, tag="gps", bufs=1)
            nc.tensor.matmul(lgps[:], ones_t[:], gb2v, start=True, stop=False)
            nc.tensor.matmul(lgps[:], g2_t[:], gw2v, start=False, stop=True)
            warm(3)

            exps_t = apool.tile([64, E], F32, tag="exps")
            se_t = apool.tile([64, 1], F32, tag="se")
            nc.scalar.activation(exps_t[:], lgps[:], AF.Exp, accum_out=se_t[:])
            rec_t = apool.tile([64, 1], F32, tag="rec")
            nc.vector.reciprocal(rec_t[:], se_t[:])
            coeff_t = apool.tile([64, E], BF16, tag="coeff")
            nc.vector.tensor_scalar(coeff_t[:], exps_t[:], rec_t[:], None,
                                    OP.mult)

            # coeff transpose [64,8] -> [8,64] (PSUM, read in place)
            ctps = psg.tile([E, 64], BF16, tag="ctps", bufs=1)
            nc.tensor.matmul(ctps[:], coeff_t[:], identv, is_transpose=True,
                             start=True, stop=True)
            warm(2)

            # masked coeff tensor: Ball[e, j, b] = coeffT[e,b] * mask[e,j]
            ball_t = cpool.tile([E, 14, BL], BF16, tag="ball")
            nc.vector.tensor_tensor(
                ball_t[:], ctps[:].unsqueeze(1).broadcast_to((E, 14, BL)),
                maskv, OP.mult)
            coeffT_t = cpool.tile([E, BL], BF16, tag="coeffT")
            nc.vector.tensor_copy(coeffT_t[:], ctps[:])

            # selection matmuls -> per-partition coefficient planes (x16)
            nc.tensor.matmul(sa_ps[:], selT, ball_t[:, 0:8, :],
                             start=True, stop=True)
            sb_ps = pss.tile([128, 6, BL], F32, tag="SB")
            nc.tensor.matmul(sb_ps[:, 0:4, :], selP, ball_t[:, 8:12, :],
                             start=True, stop=True)
            nc.tensor.matmul(sb_ps[:, 4:6, :], selQ, ball_t[:, 12:14, :],
                             start=True, stop=True)

            # scaled inputs straight from the PSUM selection results
            x0m_t = xpool.tile([128, E, BL], FP8, tag="x0m")
            nc.vector.tensor_tensor(
                x0m_t[:, 0:4, :], zcT0.unsqueeze(1).broadcast_to((128, 4, BL)),
                sa_ps[:, 0:4, :], OP.mult)
            nc.vector.tensor_tensor(
                x0m_t[:, 4:8, :], zcT0.unsqueeze(1).broadcast_to((128, 4, BL)),
                sa_ps[:, 4:8, :], OP.mult)
            x0tail_t = cpool.tile([128, 2, BL], FP8, tag="x0tail")
            nc.vector.tensor_tensor(
                x0tail_t[:], zt4.unsqueeze(1).broadcast_to((128, 2, BL)),
                sb_ps[:, 4:6, :], OP.mult)
            xz_t = cpool.tile([128, 4, BL], BF16, tag="xz")
            nc.vector.tensor_tensor(
                xz_t[:], zdup.unsqueeze(1).broadcast_to((128, 4, BL)),
                sb_ps[:, 0:4, :], OP.mult)

            # bf16 coefficient planes for the seam scalings; copied before
            # any warm matmul can stomp the selection PSUM
            S_t = cpool.tile([128, E, BL], BF16, tag="S")
            nc.vector.tensor_copy(S_t[:], sa_ps[:])

            x1h_t = xpool.tile([128, 4, E, BL], FP8, tag="x1h")
            x2h_t = xpool.tile([128, 4, E, BL], BF16, tag="x2h")

            # ---- layer 0: bias + 4 main DR pairs + 1 tail DR ----
            l0ps = psm.tile([64, H], F32, tag="lps")
            nc.tensor.matmul(l0ps[:], coeffT_t[:], bc0, start=True, stop=False)
            for g in range(4):
                nc.tensor.matmul(l0ps[:], x0m_t[:, 2 * g:2 * g + 2, :],
                                 w0t[:, g, :, :], start=False, stop=False,
                                 perf_mode=DR)
            nc.tensor.matmul(l0ps[:], x0tail_t[:], w0t[:, 4, :, :],
                             start=False, stop=True, perf_mode=DR)

            # ---- layer 1 bias + z-part (runs on PE during seam 0) ----
            l1ps = psm.tile([64, H], F32, tag="lps")
            nc.tensor.matmul(l1ps[:], coeffT_t[:], bc1, start=True, stop=False)
            for j in range(4):
                nc.tensor.matmul(l1ps[:], xz_t[:, j, :], w1z[:, j, :],
                                 start=False, stop=False)

            # ---- seam helper: ELU chunk -> transpose -> scaled next input
            def seam_chunk(l_ps, m, hb, hTp, xt_next):
                sl = slice(128 * m, 128 * (m + 1))
                elu(hb[:, sl], l_ps[:, sl], [64, 128], s=INV)
                nc.tensor.matmul(hTp[:, m, :], hb[:, sl], identv,
                                 is_transpose=True, start=True, stop=True)
                nc.vector.tensor_tensor(
                    xt_next[:, m, :, :],
                    hTp[:, m, :].unsqueeze(1).broadcast_to((128, E, BL)),
                    S_t[:], OP.mult)

            hb0 = apool.tile([64, 512], BF16, tag="hb")
            hTp0 = psm.tile([128, 4, BL], BF16, tag="hTp", bufs=1)
            seam_chunk(l0ps, 0, hb0, hTp0, x1h_t)
            seam_chunk(l0ps, 1, hb0, hTp0, x1h_t)

            warm(2)

            # ---- layer 1 h-part, P-outer, transposes slotted mid-block ----
            for e in range(4):
                nc.tensor.matmul(l1ps[:], x1h_t[:, 0:2, e, :],
                                 w1t[:, 0, e, :, :], start=False,
                                 stop=False, perf_mode=DR)
            seam_chunk(l0ps, 2, hb0, hTp0, x1h_t)
            for e in range(4, E):
                nc.tensor.matmul(l1ps[:], x1h_t[:, 0:2, e, :],
                                 w1t[:, 0, e, :, :], start=False,
                                 stop=False, perf_mode=DR)
            seam_chunk(l0ps, 3, hb0, hTp0, x1h_t)
            for e in range(E):
                nc.tensor.matmul(l1ps[:], x1h_t[:, 2:4, e, :],
                                 w1t[:, 1, e, :, :], start=False,
                                 stop=(e == E - 1), perf_mode=DR)

            # ---- layer 2 bias + z-part (runs on PE during seam 1) ----
            l2ps = pso.tile([BL, OUT], F32, tag="ops")
            nc.tensor.matmul(l2ps[:], coeffT_t[:], bc2, start=True, stop=False)
            for j in range(4):
                nc.tensor.matmul(l2ps[:], xz_t[:, j, :], w2z[:, j, :],
                                 start=False, stop=False)

            warm(3)

            # ---- seam 1 interleaved with layer 2 h-part (t-outer) ----
            hb1 = apool.tile([64, 512], BF16, tag="hb")
            hTp1 = psm.tile([128, 4, BL], BF16, tag="hTp", bufs=1)
            for t in range(4):
                seam_chunk(l1ps, t, hb1, hTp1, x2h_t)
                for e in range(E):
                    nc.tensor.matmul(l2ps[:], x2h_t[:, t, e, :],
                                     w2h[:, e, t, :], start=False,
                                     stop=(t == 3 and e == E - 1))


            out_t = apool.tile([BL, OUT], F32, tag="out_sb")
            nc.vector.tensor_scalar(out_t[:], l2ps[:], INV, None, OP.mult)
            nc.scalar.dma_start(out_d[:], out_t[:])

    nc.compile()
    return nc


_NC_CACHE = None


def _get_nc():
    global _NC_CACHE
    if _NC_CACHE is None:
        _NC_CACHE = _build()
    return _NC_CACHE


def _host_prep(z, c, gw0, gb0, gw1, gb1, gw2, gb2, w0, b0, w1, b1, w2, b2):
    bf = lambda a: np.ascontiguousarray(a).astype(_nbf)
    f8 = lambda a: np.ascontiguousarray(a).astype(_nf8)
    z, c = np.asarray(z), np.asarray(c)
    gw0, gw1, gw2 = np.asarray(gw0), np.asarray(gw1), np.asarray(gw2)
    w0, w1, w2 = np.asarray(w0) * WS, np.asarray(w1) * WS, np.asarray(w2) * WS
    b0, b1, b2 = np.asarray(b0), np.asarray(b1), np.asarray(b2)

    # w0q [128, 5, 2, 512]
    w0q = np.zeros((128, 5, 2, 512), dtype=np.float32)
    for g in range(4):
        for i in range(2):
            w0q[:, g, i, :] = w0[2 * g + i, 0:128, :]
    for j in range(2):
        w0q[:, 4, j, :] = w0[4 * j:4 * j + 4, 128:160, :].reshape(128, 512)

    # w1q [128, 2, E, 2, 512] (h-rows, P-major), w1z [128, 4, 512] (z-rows)
    w1q = (w1[:, 64:576, :].reshape(E, 2, 2, 128, H)
           .transpose(3, 1, 0, 2, 4))
    w1zp = (w1[:, 0:64, :].reshape(4, 2 * 64, H)
            .transpose(1, 0, 2))                       # [128, 4, 512]
    w2hp = (w2[:, 64:576, :].reshape(E, 4, 128, OUT)
            .transpose(2, 0, 1, 3))                    # [128, 8, 4, 96]
    w2zp = (w2[:, 0:64, :].reshape(4, 2 * 64, OUT)
            .transpose(1, 0, 2))                       # [128, 4, 96]
    wbp = np.concatenate([w1zp.reshape(128, -1), w2hp.reshape(128, -1),
                          w2zp.reshape(128, -1)], axis=1)

    # aux [8, 2400]
    aux = np.zeros((E, _AX_COLS), dtype=np.float32)
    ei = np.arange(E)[:, None]
    pi = np.arange(128)[None, :]
    aux[:, _AX_SELT:_AX_SELT + 128] = 1.0
    aux[:, _AX_SELP:_AX_SELP + 128] = ((ei % 2) == (pi >= 64))
    aux[:, _AX_SELQ:_AX_SELQ + 128] = ((ei % 4) == (pi // 32))
    mask = np.zeros((E, 14), dtype=np.float32)
    mask[:, 0:8] = np.eye(E) * XS
    ji = np.arange(4)[None, :]
    mask[:, 8:12] = ((ei // 2) == ji) * XS
    mask[:, 12:14] = ((ei // 4) == np.arange(2)[None, :]) * XS
    aux[:, _AX_MASK:_AX_MASK + 896] = np.repeat(
        mask, BL, axis=1).reshape(E, 896)
    aux[:, _AX_BC0:_AX_BC0 + 512] = b0 * (WS * XS)
    aux[:, _AX_BC1:_AX_BC1 + 512] = b1 * (WS * XS)
    aux[:, _AX_BC2:_AX_BC2 + 96] = b2 * (WS * XS)

    gp_base = np.zeros((128, _GP_COLS), dtype=np.float32)
    gp_base[:, _GP_GW0M:_GP_GW0M + 64] = gw0[0:128]
    gp_base[0:32, _GP_GW0T:_GP_GW0T + 64] = gw0[128:160]
    gp_base[0:64, _GP_GW1:_GP_GW1 + 64] = gw1
    gp_base[0:64, _GP_GW2:_GP_GW2 + E] = gw2
    gp_base[0, _GP_GB0:_GP_GB0 + 64] = gb0
    gp_base[0, _GP_GB1:_GP_GB1 + 64] = gb1
    gp_base[0, _GP_GB2:_GP_GB2 + 8] = gb2
    gp_base[0:64, _GP_ID:_GP_ID + 64] = np.eye(64, dtype=np.float32)

    shared = {
        "aux": bf(aux),
        "w0q": f8(w0q.reshape(128, -1)),
        "w1q": f8(w1q.reshape(128, -1)),
        "wb": bf(wbp),
    }
    zc = np.concatenate([z, c], axis=1)  # [B, 160]
    in_maps = []
    for i in range(NCORES):
        gpi = gp_base.copy()
        zcT = zc[i * BL:(i + 1) * BL, :].T  # [160, 64]
        gpi[:, _GP_ZCT0:_GP_ZCT0 + 64] = zcT[0:128]
        gpi[0:64, _GP_ZDUP:_GP_ZDUP + 64] = zcT[0:64]
        gpi[64:128, _GP_ZDUP:_GP_ZDUP + 64] = zcT[0:64]
        gpi[:, _GP_ZT4:_GP_ZT4 + 64] = np.tile(zcT[128:160], (4, 1))
        gpi[0:32, _GP_ZCT1:_GP_ZCT1 + 64] = zcT[128:160]
        m = dict(shared)
        m["gpack"] = bf(gpi)
        in_maps.append(m)
    return in_maps


def kernel(**inputs):
    nc = _get_nc()
    in_maps = _host_prep(**inputs)
    res = bass_utils.run_bass_kernel_spmd(nc, in_maps,
                                          core_ids=list(range(NCORES)))
    return np.concatenate([r["out"] for r in res.results], axis=0)
